# revision 1
# baseline (speedup 1.0000x reference)
"""Trainium2 Bass kernel for nn_MultiHeadAttention_59614146068609.

Sharding: 8 cores = 2 batches x 4 head-groups (4 heads each).
Each core projects q/k/v for its batch with its head-slice of Wq/Wk/Wv
(column-sharded), runs causal+padded attention for its 4 heads, and
applies its row-slice of Wo, producing a partial [D, S] output. The host
sums the 4 partials per batch and adds bo.

All matmuls run as float32r (2 cycles/row PE mode, ~1e-4 rel err).
Layout trick: scores are computed transposed (S.T[k, q], k on
partitions) so softmax sums come from an appended ones-column of V and
no on-chip transposes are needed anywhere.

The kernel is specialized at build time on kb_cap = number of 128-wide
key blocks that contain any unpadded key (derived from the runtime
key_padding_mask); fully padded key blocks contribute exactly zero
attention weight, so their projection/QK/exp/PV work is skipped.
"""

import numpy as np

S = 2048
B = 2
D = 1024
H = 16
DK = 64
N_CORES = 8
GROUPS = N_CORES // B          # head groups per batch = 4
HPG = H // GROUPS              # heads per group = 4
OC = HPG * DK                  # per-core projected dim = 256
OT = OC // 128                 # o-tiles per core = 2
IT = D // 128                  # contraction tiles = 8
SC = S // 512                  # sequence chunks of 512 = 4
KB = S // 128                  # k blocks of 128 = 16
NEG = -1e30

_cache = {}


def _build_nc(kb_cap):
    import concourse.bacc as bacc
    import concourse.bass as bass
    import concourse.mybir as mybir
    import concourse.tile as tile
    from concourse import library_config

    F32 = mybir.dt.float32
    F32R = mybir.dt.float32r
    FP16 = mybir.dt.float16
    Exp = mybir.ActivationFunctionType.Exp
    Identity = mybir.ActivationFunctionType.Identity
    PSUM = bass.MemorySpace.PSUM

    ksc = -(-kb_cap * 128 // 512)        # 512-chunks of k_T to project
    vrounds = [
        range(r * 8, min(kb_cap, (r + 1) * 8)) for r in range(-(-kb_cap // 8))
    ]

    nc = bacc.Bacc("TRN2", target_bir_lowering=False, debug=False)

    xq = nc.dram_tensor("xq", [D, S], FP16, kind="ExternalInput")
    xk = nc.dram_tensor("xk", [D, S], FP16, kind="ExternalInput")
    xv = nc.dram_tensor("xv", [D, S], FP16, kind="ExternalInput")
    wq = nc.dram_tensor("wq", [D, OC], FP16, kind="ExternalInput")
    wk = nc.dram_tensor("wk", [D, OC], FP16, kind="ExternalInput")
    wv = nc.dram_tensor("wv", [D, OC], FP16, kind="ExternalInput")
    wo = nc.dram_tensor("wo", [OC, D], FP16, kind="ExternalInput")
    bias_qk = nc.dram_tensor("bias_qk", [128, 2, OT], F32, kind="ExternalInput")
    bias_v = nc.dram_tensor("bias_v", [1, OC], F32, kind="ExternalInput")
    pad = nc.dram_tensor("pad", [128, KB], F32, kind="ExternalInput")
    causal = nc.dram_tensor("causal", [128, 128], FP16, kind="ExternalInput")
    out_t = nc.dram_tensor("out_t", [D, S], F32, kind="ExternalOutput")

    with tile.TileContext(nc) as tc, nc.allow_low_precision(
        reason="fp32r compute throughout; validated vs fp64 reference"
    ):
        with (
            tc.tile_pool(name="persist", bufs=1) as pp,
            tc.tile_pool(name="xs", bufs=6) as xs,
        ):
            nc.gpsimd.load_library(library_config.attn)

            # ---- persistent SBUF tensors ----
            t_wq = pp.tile([128, IT, OC], FP16)
            t_wk = pp.tile([128, IT, OC], FP16)
            t_wv = pp.tile([128, IT, OC], FP16)
            t_wo = pp.tile([128, OT, D], FP16)
            t_bqk = pp.tile([128, 2, OT], F32)
            t_bv = pp.tile([128, OC], F32)
            t_pad = pp.tile([128, KB], F32)
            t_causal = pp.tile([128, 128], FP16)
            t_qT = pp.tile([128, HPG, S], FP16)
            t_kT = pp.tile([128, HPG, ksc * 512], FP16)
            t_V = pp.tile([128, kb_cap, HPG, 128], FP16)
            t_OT = pp.tile([128, OT, S], FP16)

            nc.scalar.dma_start(out=t_wq, in_=wq[:].rearrange("(i p) o -> p i o", p=128))
            nc.scalar.dma_start(out=t_wk, in_=wk[:].rearrange("(i p) o -> p i o", p=128))
            nc.scalar.dma_start(out=t_wv, in_=wv[:].rearrange("(i p) o -> p i o", p=128))
            nc.scalar.dma_start(out=t_bqk, in_=bias_qk[:])
            nc.scalar.dma_start(out=t_pad, in_=pad[:])
            nc.scalar.dma_start(out=t_causal, in_=causal[:])
            # broadcast the v bias across partitions once (free dim = o)
            t_bv1 = pp.tile([1, OC], F32)
            nc.scalar.dma_start(out=t_bv1, in_=bias_v[:])
            nc.scalar.dma_start(out=t_wo, in_=wo[:].rearrange("(j p) d -> p j d", p=128))
            nc.gpsimd.partition_broadcast(t_bv, t_bv1)
            nc.gpsimd.memset(t_qT[64:128, :, :], 0)
            nc.gpsimd.memset(t_kT[64:128, :, :], 0)
            nc.gpsimd.memset(t_V[:], 0)
            nc.vector.memset(t_V[:, :, :, DK : DK + 1], 1.0)

            # ---- phase A: projections ----
            # q and k land transposed ([o, s], o on partitions); v lands
            # natural ([s, o], s on partitions) for the PV matmul.
            with tc.tile_pool(name="ps_proj", bufs=8, space=PSUM) as ps_proj:
                for name, xin, w_sb, nsc in (("q", xq, t_wq, SC), ("k", xk, t_wk, ksc)):
                    dst = t_qT if name == "q" else t_kT
                    bidx = 0 if name == "q" else 1
                    acc = [
                        ps_proj.tile(
                            [128, 512], F32, tag="proj", name=f"acc_{name}_{n}"
                        )
                        for n in range(OT * nsc)
                    ]
                    xts = []
                    for i in range(IT):
                        xt = xs.tile(
                            [128, nsc * 512], FP16, tag=f"x{name}",
                            name=f"xt_{name}_{i}", bufs=IT,
                        )
                        nc.sync.dma_start(
                            out=xt,
                            in_=xin[i * 128 : (i + 1) * 128, 0 : nsc * 512],
                        )
                        xts.append(xt)
                    for sc in range(nsc):
                        for ot in range(OT):
                            for i in range(IT):
                                nc.tensor.matmul(
                                    acc[ot * nsc + sc],
                                    w_sb[:, i, ot * 128 : (ot + 1) * 128],
                                    xts[i][:, sc * 512 : (sc + 1) * 512],
                                    start=(i == 0),
                                    stop=(i == IT - 1),
                                )
                    for ot in range(OT):
                        for sc in range(nsc):
                            for half in range(2):
                                h = 2 * ot + half
                                p0 = half * 64
                                nc.vector.tensor_scalar_add(
                                    out=dst[0:64, h, sc * 512 : (sc + 1) * 512],
                                    in0=acc[ot * nsc + sc][p0 : p0 + 64, :],
                                    scalar1=t_bqk[p0 : p0 + 64, bidx, ot : ot + 1],
                                )

                # v natural: lhsT = x tile (stationary), rhs = wv (moving).
                # One accumulation group per psum bank (interleaving two
                # start/accumulate groups in one bank corrupts has_written).
                for rnd, sts in enumerate(vrounds):
                    sts = list(sts)
                    w = len(sts) * 128
                    vacc = [
                        ps_proj.tile([128, OC], F32, tag="proj", name=f"vacc_{rnd}_{n}")
                        for n in range(len(sts))
                    ]
                    for i in range(IT):
                        xt = xs.tile([128, w], FP16, tag="xv", name=f"xtv_{rnd}_{i}", bufs=3)
                        nc.sync.dma_start(
                            out=xt,
                            in_=xv[
                                i * 128 : (i + 1) * 128,
                                sts[0] * 128 : sts[0] * 128 + w,
                            ],
                        )
                        for n in range(len(sts)):
                            nc.tensor.matmul(
                                vacc[n],
                                xt[:, n * 128 : (n + 1) * 128],
                                t_wv[:, i, :],
                                start=(i == 0),
                                stop=(i == IT - 1),
                            )
                    for n, st in enumerate(sts):
                        nc.vector.tensor_add(
                            out=t_V[:, st, :, 0:DK],
                            in0=vacc[n].rearrange("p (h d) -> p h d", h=HPG),
                            in1=t_bv.rearrange("p (h d) -> p h d", h=HPG),
                        )

            # ---- phase B: attention (S.T layout) + interleaved phase C ----
            with (
                tc.tile_pool(name="ps_att", bufs=3, space=PSUM) as ps_att,
                tc.tile_pool(name="ps_o", bufs=3, space=PSUM) as ps_o,
                tc.tile_pool(name="ps_c", bufs=2, space=PSUM) as ps_c,
                tc.tile_pool(name="pb", bufs=4) as pb,
                tc.tile_pool(name="nrm", bufs=2) as nrm,
                tc.tile_pool(name="stg", bufs=4) as stg,
            ):
                for qc in range(SC):
                    q0 = qc * 512
                    nkb = min(4 * (qc + 1), kb_cap)
                    for pair in ((0, 1), (2, 3)):
                        o_ps = {
                            h: ps_o.tile(
                                [128, 512], F32, tag="ops", name=f"ops_{qc}_{h}"
                            )
                            for h in pair
                        }
                        for kb in range(nkb):
                            k0 = kb * 128
                            off = max(0, k0 - q0)
                            st = {}
                            for h in pair:
                                st[h] = ps_att.tile(
                                    [128, 512], F32, tag="st", name=f"st_{qc}_{h}_{kb}"
                                )
                                nc.tensor.matmul(
                                    st[h][:, off:512],
                                    t_kT[:, h, k0 : k0 + 128],
                                    t_qT[:, h, q0 + off : q0 + 512],
                                    start=True,
                                    stop=True,
                                )
                            for h in pair:
                                if k0 >= q0:
                                    nc.vector.tensor_add(
                                        out=st[h][:, off : off + 128],
                                        in0=st[h][:, off : off + 128],
                                        in1=t_causal,
                                    )
                                pt = pb.tile(
                                    [128, 512], FP16, tag="pt", name=f"pt_{qc}_{h}_{kb}"
                                )
                                nc.scalar.activation(
                                    out=pt[:, off:512],
                                    in_=st[h][:, off:512],
                                    func=Exp,
                                    bias=t_pad[:, kb : kb + 1],
                                    scale=1.0,
                                )
                                nc.tensor.matmul(
                                    o_ps[h][:, off:512],
                                    t_V[:, kb, h, :],
                                    pt[:, off:512],
                                    start=(kb == 0),
                                    stop=(kb == nkb - 1),
                                )
                        for h in pair:
                            ot, p0 = h // 2, (h % 2) * 64
                            t_l = nrm.tile([128, 512], F32, tag="l", name=f"l_{qc}_{h}")
                            nc.vector.tensor_copy(
                                t_l[0:1, :], o_ps[h][DK : DK + 1, :]
                            )
                            t_r = nrm.tile([128, 512], F32, tag="r", name=f"r_{qc}_{h}")
                            nc.vector.reciprocal_approx_fast(t_r[0:1, :], t_l[0:1, :])
                            t_rb = nrm.tile([DK, 512], F32, tag="rb", name=f"rb_{qc}_{h}")
                            nc.gpsimd.partition_broadcast(t_rb, t_r[0:1, :])
                            nc.vector.tensor_mul(
                                t_OT[p0 : p0 + DK, ot, q0 : q0 + 512],
                                o_ps[h][0:DK, :],
                                t_rb,
                            )
                    # phase C for this 512-chunk of s (needs all 4 heads)
                    for dt_ in range(D // 128):
                        ops = ps_c.tile([128, 512], F32, tag="c", name=f"c_{qc}_{dt_}")
                        for j in range(OT):
                            nc.tensor.matmul(
                                ops,
                                t_wo[:, j, dt_ * 128 : (dt_ + 1) * 128],
                                t_OT[:, j, q0 : q0 + 512],
                                start=(j == 0),
                                stop=(j == OT - 1),
                            )
                        st_o = stg.tile([128, 512], F32, tag="s", name=f"so_{qc}_{dt_}")
                        nc.vector.tensor_copy(st_o, ops)
                        nc.sync.dma_start(
                            out=out_t[dt_ * 128 : (dt_ + 1) * 128, q0 : q0 + 512],
                            in_=st_o,
                        )
    nc.compile()
    return nc


def _get_nc(kb_cap):
    key = ("nc", kb_cap)
    if key not in _cache:
        _cache[key] = _build_nc(kb_cap)
    return _cache[key]


def kernel(
    query,
    key,
    value,
    Wq,
    bq,
    Wk,
    bk,
    Wv,
    bv,
    Wo,
    bo,
    attn_mask,
    key_padding_mask,
):
    import ml_dtypes
    from concourse import bass_utils

    query = np.asarray(query, dtype=np.float32)
    key = np.asarray(key, dtype=np.float32)
    value = np.asarray(value, dtype=np.float32)
    Wq = np.asarray(Wq, dtype=np.float32)
    bq = np.asarray(bq, dtype=np.float32)
    Wk = np.asarray(Wk, dtype=np.float32)
    bk = np.asarray(bk, dtype=np.float32)
    Wv = np.asarray(Wv, dtype=np.float32)
    bv = np.asarray(bv, dtype=np.float32)
    Wo = np.asarray(Wo, dtype=np.float32)
    bo = np.asarray(bo, dtype=np.float32)
    attn_mask = np.asarray(attn_mask)
    key_padding_mask = np.asarray(key_padding_mask)

    # this kernel hardcodes the causal structure of attn_mask
    expected = np.triu(np.ones((S, S), dtype=bool), k=1)
    assert np.array_equal(attn_mask, expected), "kernel assumes causal attn_mask"

    # number of 128-blocks that contain any valid (unpadded) key
    valid = ~key_padding_mask  # [B, S]
    kb_cap = 0
    for b in range(B):
        nz = np.nonzero(valid[b])[0]
        cap = (int(nz.max()) // 128 + 1) if nz.size else 1
        kb_cap = max(kb_cap, cap)

    scale = np.float32(1.0 / np.sqrt(DK))
    causal_tile = np.where(
        np.arange(128)[None, :] >= np.arange(128)[:, None], 0.0, -60000.0
    ).astype(np.float16)

    # per-batch transposed activations (shared by the batch's 4 cores)
    xq_b = [np.ascontiguousarray(query[:, b, :].T.astype(np.float16)) for b in range(B)]
    xk_b = [np.ascontiguousarray(key[:, b, :].T.astype(np.float16)) for b in range(B)]
    xv_b = [np.ascontiguousarray(value[:, b, :].T.astype(np.float16)) for b in range(B)]
    pad_b = [
        np.ascontiguousarray(
            np.where(key_padding_mask[b], NEG, 0.0)
            .astype(np.float32)
            .reshape(KB, 128)
            .T
        )
        for b in range(B)
    ]

    in_maps = []
    for c in range(N_CORES):
        b = c // GROUPS
        g = c % GROUPS
        o0 = g * OC
        osl = slice(o0, o0 + OC)
        bias_qk = np.stack(
            [
                (bq[osl] * scale).reshape(OT, 128).T,
                bk[osl].reshape(OT, 128).T,
            ],
            axis=1,
        ).astype(np.float32)  # [128, 2, OT]
        in_maps.append(
            {
                "xq": xq_b[b],
                "xk": xk_b[b],
                "xv": xv_b[b],
                "wq": np.ascontiguousarray((Wq[osl, :] * scale).T.astype(np.float16)),
                "wk": np.ascontiguousarray(Wk[osl, :].T.astype(np.float16)),
                "wv": np.ascontiguousarray(Wv[osl, :].T.astype(np.float16)),
                "wo": np.ascontiguousarray(Wo[:, osl].T).astype(np.float16),
                "bias_qk": np.ascontiguousarray(bias_qk),
                "bias_v": np.ascontiguousarray(bv[osl][None, :]),
                "pad": pad_b[b],
                "causal": causal_tile,
            }
        )

    res = bass_utils.run_bass_kernel_spmd(
        _get_nc(kb_cap), in_maps, core_ids=list(range(N_CORES))
    )
    _cache["last_res"] = res

    out = np.zeros((S, B, D), dtype=np.float32)
    for b in range(B):
        acc = np.zeros((D, S), dtype=np.float32)
        for g in range(GROUPS):
            acc += res.results[b * GROUPS + g]["out_t"]
        out[:, b, :] = acc.T + bo[None, :]
    return out



# revision 17
# speedup vs baseline: 1.0455x; 1.0455x over previous
"""Trainium2 Bass kernel for nn_MultiHeadAttention_59614146068609.

Sharding: 8 cores = 2 batches x 4 head-groups (4 heads each). Each core
projects q/k/v for its batch with its head-slice of Wq/Wk/Wv
(column-sharded), runs causal+padded attention for its 4 heads, and
applies its row-slice of Wo, producing a partial [D, S] fp16 output.
The host sums the 4 partials per batch and adds bo.

Schedule: single software-pipelined pass. Attention is ACT(exp)-paced,
so projection and output (Wo) matmul groups are injected as PE filler
between attention steps; the PE stays busy while the scalar engine
churns exp.

Key layout choices:
 - scores computed transposed (S.T[k, q], k on partitions); softmax
   denominator comes from an appended ones-column of V.
 - heads processed in pairs: qT/kT hold a head pair stacked on
   partitions (64+64); QK runs as two concurrent row-tiled matmuls
   (tile_position (0,0)/(64,0)), scores for the pair land in one
   2-bank PSUM tile and one ACTIVATE(exp) covers both heads.
 - key-padding folded into V: padded V rows are zeroed and the
   ones-column holds the valid mask, so exp needs no per-block bias
   and masked keys contribute exactly zero weight and zero denominator.

Specialized at build time on kb_cap = number of 128-wide key blocks
containing any unpadded key.
"""

import numpy as np

S = 2048
B = 2
D = 1024
H = 16
DK = 64
N_CORES = 8
GROUPS = N_CORES // B          # head groups per batch = 4
HPG = H // GROUPS              # heads per group = 4
OC = HPG * DK                  # per-core projected dim = 256
OT = OC // 128                 # head pairs per core = 2
IT = D // 128                  # contraction tiles = 8
SC = S // 512                  # sequence chunks of 512 = 4
KB = S // 128                  # k blocks of 128 = 16

_cache = {}


def _build_nc(kb_cap):
    import concourse.bacc as bacc
    import concourse.bass as bass
    import concourse.mybir as mybir
    import concourse.tile as tile
    from concourse import library_config

    F32 = mybir.dt.float32
    FP16 = mybir.dt.float16
    Exp = mybir.ActivationFunctionType.Exp
    PSUM = bass.MemorySpace.PSUM

    ksc = -(-kb_cap * 128 // 512)        # 512-chunks of k to project
    KW = ksc * 512
    VW = kb_cap * 128

    nc = bacc.Bacc("TRN2", target_bir_lowering=False, debug=False)

    xq = nc.dram_tensor("xq", [D, S], FP16, kind="ExternalInput")
    xk = nc.dram_tensor("xk", [D, S], FP16, kind="ExternalInput")
    xv = nc.dram_tensor("xv", [D, S], FP16, kind="ExternalInput")
    wq = nc.dram_tensor("wq", [128, IT * OC], FP16, kind="ExternalInput")
    wk = nc.dram_tensor("wk", [128, IT * OC], FP16, kind="ExternalInput")
    wv = nc.dram_tensor("wv", [128, IT * OC], FP16, kind="ExternalInput")
    wo = nc.dram_tensor("wo", [128, OT * D], FP16, kind="ExternalInput")
    # konst f32: [0:2]=bias_q(pair), [2:4]=bias_k, [4:20]=vmask, [20:276]=bv
    konst = nc.dram_tensor("konst", [128, 20 + OC], F32, kind="ExternalInput")
    # konst16 fp16: [0:64]=vmask4 (st-major), [64:320]=causal2
    konst16 = nc.dram_tensor("konst16", [128, KB * HPG + 256], FP16,
                             kind="ExternalInput")
    out_t = nc.dram_tensor("out_t", [D, S], FP16, kind="ExternalOutput")

    with tile.TileContext(nc) as tc, nc.allow_low_precision(
        reason="fp16 compute throughout; validated vs fp64 reference"
    ):
        with (
            tc.tile_pool(name="persist", bufs=1) as pp,
            tc.tile_pool(name="xs", bufs=1) as xs,
            tc.tile_pool(name="pt", bufs=3) as ptp,
            tc.tile_pool(name="nrm", bufs=2) as nrmp,
            tc.tile_pool(name="stg", bufs=3) as stgp,
            tc.tile_pool(name="vtmp", bufs=2) as vtp,
            tc.tile_pool(name="ps_st", bufs=2, space=PSUM) as ps_st,
            tc.tile_pool(name="ps_o", bufs=2, space=PSUM) as ps_o,
            tc.tile_pool(name="ps_w", bufs=2, space=PSUM) as ps_w,
        ):
            nc.gpsimd.load_library(library_config.attn)

            # ---- persistent SBUF tensors ----
            t_wq = pp.tile([128, IT, OC], FP16)
            t_wk = pp.tile([128, IT, OC], FP16)
            t_wv = pp.tile([128, IT, OC], FP16)
            t_wo = pp.tile([128, OT, D], FP16)
            t_k32 = pp.tile([128, 20 + OC], F32)
            t_k16 = pp.tile([128, KB * HPG + 256], FP16)
            t_qT = pp.tile([128, OT, S], FP16)
            t_kT = pp.tile([128, OT, KW], FP16)
            t_V = pp.tile([128, kb_cap, HPG, 128], FP16)
            t_OT = pp.tile([128, OT, S], FP16)

            xq_t = [xs.tile([128, S], FP16, tag="xq", name=f"xq{i}", bufs=IT)
                    for i in range(IT)]
            xk_t = [xs.tile([128, KW], FP16, tag="xk", name=f"xk{i}", bufs=IT)
                    for i in range(IT)]
            xv_t = [xs.tile([128, VW], FP16, tag="xv", name=f"xv{i}", bufs=IT)
                    for i in range(IT)]

            # ---- input DMAs; only SP/Activation/GpSimd queues can start DMAs.
            # Spread across the three; first-needed tensors first per queue.
            nc.sync.dma_start(out=t_k32, in_=konst[:])
            nc.sync.dma_start(out=t_k16, in_=konst16[:])
            nc.sync.dma_start(out=t_wq, in_=wq[:].rearrange("p (i o) -> p i o", i=IT))
            for i in range(IT):
                nc.sync.dma_start(out=xq_t[i], in_=xq[i * 128:(i + 1) * 128, :])
            nc.scalar.dma_start(out=t_wk, in_=wk[:].rearrange("p (i o) -> p i o", i=IT))
            for i in range(IT):
                nc.scalar.dma_start(out=xk_t[i], in_=xk[i * 128:(i + 1) * 128, 0:KW])
            nc.gpsimd.dma_start(out=t_wv, in_=wv[:].rearrange("p (i o) -> p i o", i=IT))
            for i in range(IT):
                nc.gpsimd.dma_start(out=xv_t[i], in_=xv[i * 128:(i + 1) * 128, 0:VW])
            nc.gpsimd.dma_start(
                out=t_wo, in_=wo[:].rearrange("p (j d) -> p j d", j=OT))

            # early dummy exp: pull the ACT table load into the startup window
            nc.scalar.activation(
                out=t_OT[0:1, 0, 0:1], in_=t_k32[0:1, 0:1], func=Exp)

            # ---- work-unit generators ----
            def qk_proj_group(which, pair, sc):
                w_sb = t_wq if which == "q" else t_wk
                xts = xq_t if which == "q" else xk_t
                dst = t_qT if which == "q" else t_kT
                bidx = 0 if which == "q" else 1
                acc = ps_w.tile([128, 512], F32, tag="w",
                                name=f"acc_{which}_{pair}_{sc}")
                for i in range(IT):
                    nc.tensor.matmul(
                        acc,
                        w_sb[:, i, pair * 128:(pair + 1) * 128],
                        xts[i][:, sc * 512:(sc + 1) * 512],
                        start=(i == 0),
                        stop=(i == IT - 1),
                    )
                nc.vector.tensor_scalar_add(
                    out=dst[:, pair, sc * 512:(sc + 1) * 512],
                    in0=acc,
                    scalar1=t_k32[:, bidx * 2 + pair:bidx * 2 + pair + 1],
                )

            def v_proj_group(st):
                vacc = ps_w.tile([128, 512], F32, tag="w", name=f"vacc_{st}")
                for i in range(IT):
                    nc.tensor.matmul(
                        vacc[:, 0:OC],
                        xv_t[i][:, st * 128:(st + 1) * 128],
                        t_wv[:, i, :],
                        start=(i == 0),
                        stop=(i == IT - 1),
                    )
                tmp = vtp.tile([128, OC], F32, tag="vt", name=f"vt_{st}", bufs=2)
                nc.vector.tensor_add(out=tmp, in0=vacc[:, 0:OC], in1=t_k32[:, 20:20 + OC])
                nc.vector.tensor_scalar_mul(
                    out=t_V[:, st, :, 0:DK],
                    in0=tmp.rearrange("p (h d) -> p h d", h=HPG),
                    scalar1=t_k32[:, 4 + st:5 + st],
                )
                # ones-column of V = valid-key mask for this block
                nc.vector.tensor_copy(
                    t_V[:, st, :, DK:DK + 1],
                    t_k16[:, st * HPG:(st + 1) * HPG].rearrange(
                        "p (h o) -> p h o", o=1),
                )

            def c_group(qc, dt):
                q0 = qc * 512
                pc = ps_w.tile([128, 512], F32, tag="w", name=f"c_{qc}_{dt}")
                for j in range(OT):
                    nc.tensor.matmul(
                        pc,
                        t_wo[:, j, dt * 128:(dt + 1) * 128],
                        t_OT[:, j, q0:q0 + 512],
                        start=(j == 0),
                        stop=(j == OT - 1),
                    )
                so = stgp.tile([128, 512], FP16, tag="so", name=f"so_{qc}_{dt}")
                nc.vector.tensor_copy(so, pc)
                nc.sync.dma_start(
                    out=out_t[dt * 128:(dt + 1) * 128, q0:q0 + 512], in_=so)

            # filler queue: (deadline_qc, cost_estimate, fn)
            fill = []
            for sc in range(SC):
                for pair in range(OT):
                    fill.append((sc, 1.7, lambda p=pair, s=sc: qk_proj_group("q", p, s)))
            for sc in range(ksc):
                dl = max(0, (sc * 512) // 512)  # needed for attn(qc>=sc)
                for pair in range(OT):
                    fill.append((dl, 1.7, lambda p=pair, s=sc: qk_proj_group("k", p, s)))
            for st in range(kb_cap):
                fill.append((st // 4, 0.9, lambda s=st: v_proj_group(s)))
            # order by deadline so flush/pacing pops prerequisites first
            fill.sort(key=lambda e: e[0])

            debt = [0.0]

            def maybe_fill(budget):
                debt[0] += budget
                while fill and debt[0] >= fill[0][1]:
                    _, cost, fn = fill.pop(0)
                    fn()
                    debt[0] -= cost

            def flush(qc):
                while fill and fill[0][0] <= qc:
                    _, _, fn = fill.pop(0)
                    fn()
                debt[0] = 0.0

            def attn_pair(qc, pair):
                q0 = qc * 512
                nkb = min(4 * (qc + 1), kb_cap)
                o_ps = [
                    ps_o.tile([128, 512], F32, tag="o", name=f"o_{qc}_{pair}_{a}")
                    for a in range(2)
                ]
                pts = {}

                def qk_exp(kb):
                    k0 = kb * 128
                    off = max(0, k0 - q0)
                    st = ps_st.tile([128, 1024], F32, tag="st",
                                    name=f"st_{qc}_{pair}_{kb}")
                    for a in range(2):
                        nc.tensor.matmul(
                            st[:, a * 512 + off:(a + 1) * 512],
                            t_kT[a * 64:(a + 1) * 64, pair, k0:k0 + 128],
                            t_qT[a * 64:(a + 1) * 64, pair, q0 + off:q0 + 512],
                            start=True,
                            stop=True,
                        )
                    if k0 >= q0:  # diagonal block: causal fix for both heads
                        for a in range(2):
                            nc.vector.tensor_add(
                                out=st[:, a * 512 + off:a * 512 + off + 128],
                                in0=st[:, a * 512 + off:a * 512 + off + 128],
                                in1=t_k16[:, KB * HPG:KB * HPG + 128],
                            )
                    pt = ptp.tile([128, 1024], FP16, tag="pt",
                                  name=f"pt_{qc}_{pair}_{kb}")
                    nc.scalar.activation(out=pt, in_=st, func=Exp)
                    pts[kb] = pt

                def pv(kb):
                    k0 = kb * 128
                    off = max(0, k0 - q0)
                    pt = pts.pop(kb)
                    for a in range(2):
                        nc.tensor.matmul(
                            o_ps[a][0:DK + 1, off:512],
                            t_V[:, kb, 2 * pair + a, 0:DK + 1],
                            pt[:, a * 512 + off:(a + 1) * 512],
                            start=(kb == 0),
                            stop=(kb == nkb - 1),
                        )

                qk_exp(0)
                for kb in range(1, nkb):
                    qk_exp(kb)
                    maybe_fill(0.55)
                    pv(kb - 1)
                maybe_fill(0.55)
                pv(nkb - 1)

                for a in range(2):
                    t_l = nrmp.tile([1, 512], F32, tag="l", name=f"l_{qc}_{pair}_{a}")
                    nc.vector.tensor_copy(t_l, o_ps[a][DK:DK + 1, :])
                    r = nrmp.tile([1, 512], F32, tag="r", name=f"r_{qc}_{pair}_{a}")
                    nc.vector.reciprocal_approx_fast(r, t_l)
                    rb = nrmp.tile([DK, 512], F32, tag="rb",
                                   name=f"rb_{qc}_{pair}_{a}")
                    nc.gpsimd.partition_broadcast(rb, r)
                    nc.vector.tensor_mul(
                        t_OT[a * 64:(a + 1) * 64, pair, q0:q0 + 512],
                        o_ps[a][0:DK, :],
                        rb,
                    )

            # ---- main pipeline ----
            for qc in range(SC):
                flush(qc)
                for pair in range(OT):
                    attn_pair(qc, pair)
                for dt in range(D // 128):
                    fill.append((SC + 1, 0.9, lambda q=qc, d=dt: c_group(q, d)))
            while fill:
                fill.pop(0)[2]()

    nc.compile()
    return nc


def _get_nc(kb_cap):
    key = ("nc", kb_cap)
    if key not in _cache:
        _cache[key] = _build_nc(kb_cap)
    return _cache[key]


def kernel(
    query,
    key,
    value,
    Wq,
    bq,
    Wk,
    bk,
    Wv,
    bv,
    Wo,
    bo,
    attn_mask,
    key_padding_mask,
):
    from concourse import bass_utils

    query = np.asarray(query, dtype=np.float32)
    key = np.asarray(key, dtype=np.float32)
    value = np.asarray(value, dtype=np.float32)
    Wq = np.asarray(Wq, dtype=np.float32)
    bq = np.asarray(bq, dtype=np.float32)
    Wk = np.asarray(Wk, dtype=np.float32)
    bk = np.asarray(bk, dtype=np.float32)
    Wv = np.asarray(Wv, dtype=np.float32)
    bv = np.asarray(bv, dtype=np.float32)
    Wo = np.asarray(Wo, dtype=np.float32)
    bo = np.asarray(bo, dtype=np.float32)
    attn_mask = np.asarray(attn_mask)
    key_padding_mask = np.asarray(key_padding_mask)

    # this kernel hardcodes the causal structure of attn_mask
    expected = np.triu(np.ones((S, S), dtype=bool), k=1)
    assert np.array_equal(attn_mask, expected), "kernel assumes causal attn_mask"

    # number of 128-blocks that contain any valid (unpadded) key
    valid = ~key_padding_mask  # [B, S]
    kb_cap = 0
    for b in range(B):
        nz = np.nonzero(valid[b])[0]
        cap = (int(nz.max()) // 128 + 1) if nz.size else 1
        kb_cap = max(kb_cap, cap)

    scale = np.float32(1.0 / np.sqrt(DK))
    ctile = np.where(
        np.arange(128)[None, :] >= np.arange(128)[:, None], 0.0, -60000.0
    ).astype(np.float16)
    causal2 = np.ascontiguousarray(np.concatenate([ctile, ctile], axis=1))

    def pack_w(w):  # [D, OC] -> [128, IT*OC] p-major
        return np.ascontiguousarray(
            w.reshape(IT, 128, OC).transpose(1, 0, 2).reshape(128, IT * OC)
        ).astype(np.float16)

    # per-batch transposed activations (shared by the batch's 4 cores)
    xq_b = [np.ascontiguousarray(query[:, b, :].T.astype(np.float16)) for b in range(B)]
    xk_b = [np.ascontiguousarray(key[:, b, :].T.astype(np.float16)) for b in range(B)]
    xv_b = [np.ascontiguousarray(value[:, b, :].T.astype(np.float16)) for b in range(B)]
    vm_b = [valid[b].astype(np.float32).reshape(KB, 128).T for b in range(B)]

    in_maps = []
    for c in range(N_CORES):
        b = c // GROUPS
        g = c % GROUPS
        o0 = g * OC
        osl = slice(o0, o0 + OC)
        konst = np.zeros((128, 20 + OC), np.float32)
        konst[:, 0:OT] = (bq[osl] * scale).reshape(OT, 128).T
        konst[:, 2:2 + OT] = bk[osl].reshape(OT, 128).T
        konst[:, 4:4 + KB] = vm_b[b]
        konst[:, 20:] = bv[osl][None, :]
        konst16 = np.zeros((128, KB * HPG + 256), np.float16)
        konst16[:, 0:KB * HPG] = np.repeat(
            vm_b[b].astype(np.float16)[:, :, None], HPG, axis=2
        ).reshape(128, KB * HPG)
        konst16[:, KB * HPG:] = causal2
        in_maps.append(
            {
                "xq": xq_b[b],
                "xk": xk_b[b],
                "xv": xv_b[b],
                "wq": pack_w((Wq[osl, :] * scale).T),
                "wk": pack_w(Wk[osl, :].T),
                "wv": pack_w(Wv[osl, :].T),
                "wo": np.ascontiguousarray(
                    Wo[:, osl].T.reshape(OT, 128, D).transpose(1, 0, 2)
                    .reshape(128, OT * D)).astype(np.float16),
                "konst": np.ascontiguousarray(konst),
                "konst16": np.ascontiguousarray(konst16),
            }
        )

    res = bass_utils.run_bass_kernel_spmd(
        _get_nc(kb_cap), in_maps, core_ids=list(range(N_CORES))
    )
    _cache["last_res"] = res

    out = np.zeros((S, B, D), dtype=np.float32)
    for b in range(B):
        acc = np.zeros((D, S), dtype=np.float32)
        for g in range(GROUPS):
            acc += res.results[b * GROUPS + g]["out_t"].astype(np.float32)
        out[:, b, :] = acc.T + bo[None, :]
    return out


# revision 25
# speedup vs baseline: 1.1069x; 1.0587x over previous
"""Trainium2 Bass kernel for nn_MultiHeadAttention_59614146068609.

Sharding: 8 cores = 2 batches x 4 head-groups (4 heads each). Each core
projects q/k/v for its batch with its head-slice of Wq/Wk/Wv
(column-sharded), runs causal+padded attention for its 4 heads, and
applies its row-slice of Wo, producing a partial [D, S] fp16 output.
The host sums the 4 partials per batch and adds bo.

Schedule: single software-pipelined pass. Attention is ACT(exp)-paced,
so projection and output (Wo) matmul groups are injected as PE filler
between attention steps; the PE stays busy while the scalar engine
churns exp.

Key layout choices:
 - scores computed transposed (S.T[k, q], k on partitions); softmax
   denominator comes from an appended ones-column of V.
 - heads processed in pairs: qT/kT hold a head pair stacked on
   partitions (64+64); QK runs as two concurrent row-tiled matmuls
   (tile_position (0,0)/(64,0)), scores for the pair land in one
   2-bank PSUM tile and one ACTIVATE(exp) covers both heads.
 - key-padding folded into V: padded V rows are zeroed and the
   ones-column holds the valid mask, so exp needs no per-block bias
   and masked keys contribute exactly zero weight and zero denominator.

Specialized at build time on kb_cap = number of 128-wide key blocks
containing any unpadded key.
"""

import numpy as np

S = 2048
B = 2
D = 1024
H = 16
DK = 64
N_CORES = 8
GROUPS = N_CORES // B          # head groups per batch = 4
HPG = H // GROUPS              # heads per group = 4
OC = HPG * DK                  # per-core projected dim = 256
OT = OC // 128                 # head pairs per core = 2
IT = D // 128                  # contraction tiles = 8
SC = S // 512                  # sequence chunks of 512 = 4
KB = S // 128                  # k blocks of 128 = 16

_cache = {}


def _build_nc(kb_cap):
    import concourse.bacc as bacc
    import concourse.bass as bass
    import concourse.mybir as mybir
    import concourse.tile as tile
    from concourse import library_config

    F32 = mybir.dt.float32
    FP16 = mybir.dt.float16
    Exp = mybir.ActivationFunctionType.Exp
    PSUM = bass.MemorySpace.PSUM

    ksc = -(-kb_cap * 128 // 512)        # 512-chunks of k to project
    KW = ksc * 512
    VW = kb_cap * 128

    VC = -(-kb_cap // 4)                 # 512-wide chunks of v keys

    nc = bacc.Bacc("TRN2", target_bir_lowering=False, debug=False)

    # x streams pre-packed chunk-major on host: [128, chunk, IT, 512]
    xq = nc.dram_tensor("xq", [128, SC * IT * 512], FP16, kind="ExternalInput")
    xk = nc.dram_tensor("xk", [128, ksc * IT * 512], FP16, kind="ExternalInput")
    xv = nc.dram_tensor("xv", [128, VC * IT * 512], FP16, kind="ExternalInput")
    wq = nc.dram_tensor("wq", [128, IT * OC], FP16, kind="ExternalInput")
    wk = nc.dram_tensor("wk", [128, IT * OC], FP16, kind="ExternalInput")
    wv = nc.dram_tensor("wv", [128, IT * OC], FP16, kind="ExternalInput")
    wo = nc.dram_tensor("wo", [128, OT * D], FP16, kind="ExternalInput")
    # konst f32: [0:2]=bias_q(pair), [2:4]=bias_k, [4:20]=vmask, [20:276]=bv
    konst = nc.dram_tensor("konst", [128, 20 + OC], F32, kind="ExternalInput")
    # konst16 fp16: [0:64]=vmask4 (st-major), [64:320]=causal2
    konst16 = nc.dram_tensor("konst16", [128, KB * HPG + 256], FP16,
                             kind="ExternalInput")
    out_t = nc.dram_tensor("out_t", [D, S], FP16, kind="ExternalOutput")

    with tile.TileContext(nc) as tc, nc.allow_low_precision(
        reason="fp16 compute throughout; validated vs fp64 reference"
    ):
        with (
            tc.tile_pool(name="persist", bufs=1) as pp,
            tc.tile_pool(name="pt", bufs=3) as ptp,
            tc.tile_pool(name="nrm", bufs=2) as nrmp,
            tc.tile_pool(name="stg", bufs=3) as stgp,
            tc.tile_pool(name="vtmp", bufs=2) as vtp,
            tc.tile_pool(name="ps_st", bufs=2, space=PSUM) as ps_st,
            tc.tile_pool(name="ps_o", bufs=2, space=PSUM) as ps_o,
            tc.tile_pool(name="ps_w", bufs=2, space=PSUM) as ps_w,
        ):
            nc.gpsimd.load_library(library_config.attn)

            # ---- persistent SBUF tensors ----
            t_wq = pp.tile([128, IT, OC], FP16)
            t_wk = pp.tile([128, IT, OC], FP16)
            t_wv = pp.tile([128, IT, OC], FP16)
            t_wo = pp.tile([128, OT, D], FP16)
            t_k32 = pp.tile([128, 20 + OC], F32)
            t_k16 = pp.tile([128, KB * HPG + 256], FP16)
            t_qT = pp.tile([128, OT, S], FP16)
            t_kT = pp.tile([128, OT, KW], FP16)
            t_V = pp.tile([128, kb_cap, HPG, 128], FP16)
            t_OT = pp.tile([128, OT, S], FP16)

            t_xq = pp.tile([128, SC, IT, 512], FP16)
            t_xk = pp.tile([128, ksc, IT, 512], FP16)
            t_xv = pp.tile([128, VC, IT, 512], FP16)

            # ---- input DMAs; only SP/Activation/GpSimd queues can start DMAs.
            # Only the first chunk of each x stream moves upfront; later
            # chunks are enqueued mid-program (at flush points) so they don't
            # steal HBM bandwidth from the critical startup prefix.
            CW = IT * 512

            def load_x_chunk(eng, t_x, x_dram, c):
                eng.dma_start(
                    out=t_x[:, c, :, :],
                    in_=x_dram[:, c * CW:(c + 1) * CW].rearrange(
                        "p (i s) -> p i s", i=IT),
                )

            nc.sync.dma_start(out=t_k32, in_=konst[:])
            nc.sync.dma_start(out=t_k16, in_=konst16[:])
            nc.sync.dma_start(out=t_wq, in_=wq[:].rearrange("p (i o) -> p i o", i=IT))
            load_x_chunk(nc.sync, t_xq, xq, 0)
            nc.scalar.dma_start(out=t_wk, in_=wk[:].rearrange("p (i o) -> p i o", i=IT))
            load_x_chunk(nc.scalar, t_xk, xk, 0)
            nc.gpsimd.dma_start(out=t_wv, in_=wv[:].rearrange("p (i o) -> p i o", i=IT))
            load_x_chunk(nc.gpsimd, t_xv, xv, 0)

            def load_later_chunks(qc):
                # called at flush(qc): bring in the chunks needed next
                c = qc + 1
                if c < SC:
                    load_x_chunk(nc.sync, t_xq, xq, c)
                if c < ksc:
                    load_x_chunk(nc.scalar, t_xk, xk, c)
                if c < VC:
                    load_x_chunk(nc.gpsimd, t_xv, xv, c)
                if c == 1:  # wo needed once C(0) fillers start popping
                    nc.gpsimd.dma_start(
                        out=t_wo, in_=wo[:].rearrange("p (j d) -> p j d", j=OT))

            # early dummy exp: pull the ACT table load into the startup window
            nc.scalar.activation(
                out=t_OT[0:1, 0, 0:1], in_=t_k32[0:1, 0:1], func=Exp)

            # ---- work-unit generators ----
            def qk_proj_group(which, pair, sc):
                w_sb = t_wq if which == "q" else t_wk
                xts = t_xq if which == "q" else t_xk
                dst = t_qT if which == "q" else t_kT
                bidx = 0 if which == "q" else 1
                acc = ps_w.tile([128, 512], F32, tag="w",
                                name=f"acc_{which}_{pair}_{sc}")
                for i in range(IT):
                    nc.tensor.matmul(
                        acc,
                        w_sb[:, i, pair * 128:(pair + 1) * 128],
                        xts[:, sc, i, :],
                        start=(i == 0),
                        stop=(i == IT - 1),
                    )
                nc.vector.tensor_scalar_add(
                    out=dst[:, pair, sc * 512:(sc + 1) * 512],
                    in0=acc,
                    scalar1=t_k32[:, bidx * 2 + pair:bidx * 2 + pair + 1],
                )

            def v_proj_group(st):
                vacc = ps_w.tile([128, 512], F32, tag="w", name=f"vacc_{st}")
                for i in range(IT):
                    nc.tensor.matmul(
                        vacc[:, 0:OC],
                        t_xv[:, st // 4, i, (st % 4) * 128:(st % 4 + 1) * 128],
                        t_wv[:, i, :],
                        start=(i == 0),
                        stop=(i == IT - 1),
                    )
                tmp = vtp.tile([128, OC], F32, tag="vt", name=f"vt_{st}", bufs=2)
                nc.vector.tensor_add(out=tmp, in0=vacc[:, 0:OC], in1=t_k32[:, 20:20 + OC])
                nc.vector.tensor_scalar_mul(
                    out=t_V[:, st, :, 0:DK],
                    in0=tmp.rearrange("p (h d) -> p h d", h=HPG),
                    scalar1=t_k32[:, 4 + st:5 + st],
                )
                # ones-column of V = valid-key mask for this block
                nc.vector.tensor_copy(
                    t_V[:, st, :, DK:DK + 1],
                    t_k16[:, st * HPG:(st + 1) * HPG].rearrange(
                        "p (h o) -> p h o", o=1),
                )

            def c_group(qc, dt):
                q0 = qc * 512
                pc = ps_w.tile([128, 512], F32, tag="w", name=f"c_{qc}_{dt}")
                for j in range(OT):
                    nc.tensor.matmul(
                        pc,
                        t_wo[:, j, dt * 128:(dt + 1) * 128],
                        t_OT[:, j, q0:q0 + 512],
                        start=(j == 0),
                        stop=(j == OT - 1),
                    )
                so = stgp.tile([128, 512], FP16, tag="so", name=f"so_{qc}_{dt}")
                nc.vector.tensor_copy(so, pc)
                nc.sync.dma_start(
                    out=out_t[dt * 128:(dt + 1) * 128, q0:q0 + 512], in_=so)

            # filler queue: (deadline_qc, cost_estimate, fn)
            fill = []
            for sc in range(SC):
                for pair in range(OT):
                    fill.append((sc, 1.7, lambda p=pair, s=sc: qk_proj_group("q", p, s)))
            for sc in range(ksc):
                dl = max(0, (sc * 512) // 512)  # needed for attn(qc>=sc)
                for pair in range(OT):
                    fill.append((dl, 1.7, lambda p=pair, s=sc: qk_proj_group("k", p, s)))
            for st in range(kb_cap):
                fill.append((st // 4, 0.9, lambda s=st: v_proj_group(s)))
            # order by deadline so flush/pacing pops prerequisites first
            fill.sort(key=lambda e: e[0])

            debt = [0.0]

            def maybe_fill(budget):
                debt[0] += budget
                while fill and debt[0] >= fill[0][1]:
                    _, cost, fn = fill.pop(0)
                    fn()
                    debt[0] -= cost

            def flush(qc):
                while fill and fill[0][0] <= qc:
                    _, _, fn = fill.pop(0)
                    fn()
                debt[0] = 0.0

            def attn_pair(qc, pair):
                q0 = qc * 512
                nkb = min(4 * (qc + 1), kb_cap)
                o_ps = [
                    ps_o.tile([128, 512], F32, tag="o", name=f"o_{qc}_{pair}_{a}")
                    for a in range(2)
                ]
                pts = {}

                def qk_exp(kb):
                    k0 = kb * 128
                    off = max(0, k0 - q0)
                    st = ps_st.tile([128, 1024], F32, tag="st",
                                    name=f"st_{qc}_{pair}_{kb}")
                    for a in range(2):
                        nc.tensor.matmul(
                            st[:, a * 512 + off:(a + 1) * 512],
                            t_kT[a * 64:(a + 1) * 64, pair, k0:k0 + 128],
                            t_qT[a * 64:(a + 1) * 64, pair, q0 + off:q0 + 512],
                            start=True,
                            stop=True,
                        )
                    if k0 >= q0:  # diagonal block: causal fix for both heads
                        for a in range(2):
                            nc.vector.tensor_add(
                                out=st[:, a * 512 + off:a * 512 + off + 128],
                                in0=st[:, a * 512 + off:a * 512 + off + 128],
                                in1=t_k16[:, KB * HPG:KB * HPG + 128],
                            )
                    pt = ptp.tile([128, 1024], FP16, tag="pt",
                                  name=f"pt_{qc}_{pair}_{kb}")
                    nc.scalar.activation(out=pt, in_=st, func=Exp)
                    pts[kb] = pt

                def pv(kb):
                    k0 = kb * 128
                    off = max(0, k0 - q0)
                    pt = pts.pop(kb)
                    for a in range(2):
                        nc.tensor.matmul(
                            o_ps[a][0:DK + 1, off:512],
                            t_V[:, kb, 2 * pair + a, 0:DK + 1],
                            pt[:, a * 512 + off:(a + 1) * 512],
                            start=(kb == 0),
                            stop=(kb == nkb - 1),
                        )

                qk_exp(0)
                for kb in range(1, nkb):
                    qk_exp(kb)
                    maybe_fill(0.55)
                    pv(kb - 1)
                maybe_fill(0.55)
                pv(nkb - 1)

                for a in range(2):
                    t_l = nrmp.tile([1, 512], F32, tag="l", name=f"l_{qc}_{pair}_{a}")
                    nc.vector.tensor_copy(t_l, o_ps[a][DK:DK + 1, :])
                    r = nrmp.tile([1, 512], F32, tag="r", name=f"r_{qc}_{pair}_{a}")
                    nc.vector.reciprocal_approx_fast(r, t_l)
                    rb = nrmp.tile([DK, 512], F32, tag="rb",
                                   name=f"rb_{qc}_{pair}_{a}")
                    nc.gpsimd.partition_broadcast(rb, r)
                    nc.vector.tensor_mul(
                        t_OT[a * 64:(a + 1) * 64, pair, q0:q0 + 512],
                        o_ps[a][0:DK, :],
                        rb,
                    )

            # ---- main pipeline ----
            for qc in range(SC):
                load_later_chunks(qc)
                flush(qc)
                for pair in range(OT):
                    attn_pair(qc, pair)
                for dt in range(D // 128):
                    fill.append((SC + 1, 0.9, lambda q=qc, d=dt: c_group(q, d)))
            while fill:
                fill.pop(0)[2]()

    nc.compile()
    return nc


def _get_nc(kb_cap):
    key = ("nc", kb_cap)
    if key not in _cache:
        _cache[key] = _build_nc(kb_cap)
    return _cache[key]


def kernel(
    query,
    key,
    value,
    Wq,
    bq,
    Wk,
    bk,
    Wv,
    bv,
    Wo,
    bo,
    attn_mask,
    key_padding_mask,
):
    from concourse import bass_utils

    query = np.asarray(query, dtype=np.float32)
    key = np.asarray(key, dtype=np.float32)
    value = np.asarray(value, dtype=np.float32)
    Wq = np.asarray(Wq, dtype=np.float32)
    bq = np.asarray(bq, dtype=np.float32)
    Wk = np.asarray(Wk, dtype=np.float32)
    bk = np.asarray(bk, dtype=np.float32)
    Wv = np.asarray(Wv, dtype=np.float32)
    bv = np.asarray(bv, dtype=np.float32)
    Wo = np.asarray(Wo, dtype=np.float32)
    bo = np.asarray(bo, dtype=np.float32)
    attn_mask = np.asarray(attn_mask)
    key_padding_mask = np.asarray(key_padding_mask)

    # this kernel hardcodes the causal structure of attn_mask
    expected = np.triu(np.ones((S, S), dtype=bool), k=1)
    assert np.array_equal(attn_mask, expected), "kernel assumes causal attn_mask"

    # number of 128-blocks that contain any valid (unpadded) key
    valid = ~key_padding_mask  # [B, S]
    kb_cap = 0
    for b in range(B):
        nz = np.nonzero(valid[b])[0]
        cap = (int(nz.max()) // 128 + 1) if nz.size else 1
        kb_cap = max(kb_cap, cap)

    scale = np.float32(1.0 / np.sqrt(DK))
    ctile = np.where(
        np.arange(128)[None, :] >= np.arange(128)[:, None], 0.0, -60000.0
    ).astype(np.float16)
    causal2 = np.ascontiguousarray(np.concatenate([ctile, ctile], axis=1))

    def pack_w(w):  # [D, OC] -> [128, IT*OC] p-major
        return np.ascontiguousarray(
            w.reshape(IT, 128, OC).transpose(1, 0, 2).reshape(128, IT * OC)
        ).astype(np.float16)

    # per-batch transposed activations (shared by the batch's 4 cores),
    # packed chunk-major [128, chunks*IT*512] to match the SBUF layout
    ksc = -(-kb_cap * 128 // 512)
    VC = -(-kb_cap // 4)

    def pack_x(x, b, nchunks):  # x [S, B, D] -> [128, nchunks*IT*512]
        xt = x[:, b, :].T.astype(np.float16)  # [D, S]
        xt = xt[:, 0:nchunks * 512]
        return np.ascontiguousarray(
            xt.reshape(IT, 128, nchunks, 512).transpose(1, 2, 0, 3)
            .reshape(128, nchunks * IT * 512))

    xq_b = [pack_x(query, b, SC) for b in range(B)]
    xk_b = [pack_x(key, b, ksc) for b in range(B)]
    xv_b = [pack_x(value, b, VC) for b in range(B)]
    vm_b = [valid[b].astype(np.float32).reshape(KB, 128).T for b in range(B)]

    in_maps = []
    for c in range(N_CORES):
        b = c // GROUPS
        g = c % GROUPS
        o0 = g * OC
        osl = slice(o0, o0 + OC)
        konst = np.zeros((128, 20 + OC), np.float32)
        konst[:, 0:OT] = (bq[osl] * scale).reshape(OT, 128).T
        konst[:, 2:2 + OT] = bk[osl].reshape(OT, 128).T
        konst[:, 4:4 + KB] = vm_b[b]
        konst[:, 20:] = bv[osl][None, :]
        konst16 = np.zeros((128, KB * HPG + 256), np.float16)
        konst16[:, 0:KB * HPG] = np.repeat(
            vm_b[b].astype(np.float16)[:, :, None], HPG, axis=2
        ).reshape(128, KB * HPG)
        konst16[:, KB * HPG:] = causal2
        in_maps.append(
            {
                "xq": xq_b[b],
                "xk": xk_b[b],
                "xv": xv_b[b],
                "wq": pack_w((Wq[osl, :] * scale).T),
                "wk": pack_w(Wk[osl, :].T),
                "wv": pack_w(Wv[osl, :].T),
                "wo": np.ascontiguousarray(
                    Wo[:, osl].T.reshape(OT, 128, D).transpose(1, 0, 2)
                    .reshape(128, OT * D)).astype(np.float16),
                "konst": np.ascontiguousarray(konst),
                "konst16": np.ascontiguousarray(konst16),
            }
        )

    res = bass_utils.run_bass_kernel_spmd(
        _get_nc(kb_cap), in_maps, core_ids=list(range(N_CORES))
    )
    _cache["last_res"] = res

    out = np.zeros((S, B, D), dtype=np.float32)
    for b in range(B):
        acc = np.zeros((D, S), dtype=np.float32)
        for g in range(GROUPS):
            acc += res.results[b * GROUPS + g]["out_t"].astype(np.float32)
        out[:, b, :] = acc.T + bo[None, :]
    return out


# revision 43
# speedup vs baseline: 1.1329x; 1.0235x over previous
"""Trainium2 Bass kernel for nn_MultiHeadAttention_59614146068609.

Sharding: 8 cores = 2 batches x 4 head-groups (4 heads each). Each core
projects q/k/v for its batch with its head-slice of Wq/Wk/Wv
(column-sharded), runs causal+padded attention for its 4 heads, and
applies its row-slice of Wo, producing a partial [D, S] fp16 output.
The host sums the 4 partials per batch and adds bo.

Schedule: single software-pipelined pass. Attention is ACT(exp)-paced,
so projection and output (Wo) matmul groups are injected as PE filler
between attention steps; the PE stays busy while the scalar engine
churns exp.

Key layout choices:
 - scores computed transposed (S.T[k, q], k on partitions); softmax
   denominator comes from an appended ones-column of V.
 - heads processed in pairs: qT/kT hold a head pair stacked on
   partitions (64+64); QK runs as two concurrent row-tiled matmuls
   (tile_position (0,0)/(64,0)), scores for the pair land in one
   2-bank PSUM tile and one ACTIVATE(exp) covers both heads.
 - key-padding folded into V: padded V rows are zeroed and the
   ones-column holds the valid mask, so exp needs no per-block bias
   and masked keys contribute exactly zero weight and zero denominator.

Specialized at build time on kb_cap = number of 128-wide key blocks
containing any unpadded key.
"""

import numpy as np

S = 2048
B = 2
D = 1024
H = 16
DK = 64
N_CORES = 8
GROUPS = N_CORES // B          # head groups per batch = 4
HPG = H // GROUPS              # heads per group = 4
OC = HPG * DK                  # per-core projected dim = 256
OT = OC // 128                 # head pairs per core = 2
IT = D // 128                  # contraction tiles = 8
SC = S // 512                  # sequence chunks of 512 = 4
KB = S // 128                  # k blocks of 128 = 16

_cache = {}


def _build_nc(kb_cap):
    import concourse.bacc as bacc
    import concourse.bass as bass
    import concourse.mybir as mybir
    import concourse.tile as tile
    from concourse import library_config

    F32 = mybir.dt.float32
    FP16 = mybir.dt.float16
    FP8 = mybir.dt.float8e4
    Exp = mybir.ActivationFunctionType.Exp
    PSUM = bass.MemorySpace.PSUM

    ksc = -(-kb_cap * 128 // 512)        # 512-chunks of k to project
    KW = ksc * 512
    VW = kb_cap * 128

    VC = -(-kb_cap // 4)                 # 512-wide chunks of v keys

    nc = bacc.Bacc("TRN2", target_bir_lowering=False, debug=False)

    # x streams pre-packed chunk-major on host: [128, chunk, IT, 512]
    xq = nc.dram_tensor("xq", [128, SC * IT * 512], FP16, kind="ExternalInput")
    xk = nc.dram_tensor("xk", [128, ksc * IT * 512], FP16, kind="ExternalInput")
    xv = nc.dram_tensor("xv", [128, VC * IT * 512], FP16, kind="ExternalInput")
    wq = nc.dram_tensor("wq", [128, IT * OC], FP16, kind="ExternalInput")
    wk = nc.dram_tensor("wk", [128, IT * OC], FP16, kind="ExternalInput")
    wv = nc.dram_tensor("wv", [128, IT * OC], FP16, kind="ExternalInput")
    wo = nc.dram_tensor("wo", [128, OT * D], FP16, kind="ExternalInput")
    # konst f32: [0:2]=bias_q(pair), [2:4]=bias_k, [4:20]=vmask, [20:276]=bv
    konst = nc.dram_tensor("konst", [128, 20 + OC], F32, kind="ExternalInput")
    # konst16 fp16: [0:64]=vmask4 (st-major), [64:320]=causal2
    konst16 = nc.dram_tensor("konst16", [128, KB * HPG + 256], FP16,
                             kind="ExternalInput")
    out_t = nc.dram_tensor("out_t", [D, S], FP16, kind="ExternalOutput")

    with tile.TileContext(nc) as tc, nc.allow_low_precision(
        reason="fp16 compute throughout; validated vs fp64 reference"
    ):
        with (
            tc.tile_pool(name="persist", bufs=1) as pp,
            tc.tile_pool(name="pt", bufs=3) as ptp,
            tc.tile_pool(name="nrm", bufs=2) as nrmp,
            tc.tile_pool(name="stg", bufs=3) as stgp,
            tc.tile_pool(name="vtmp", bufs=2) as vtp,
            tc.tile_pool(name="ps_st", bufs=2, space=PSUM) as ps_st,
            tc.tile_pool(name="ps_o", bufs=2, space=PSUM) as ps_o,
            tc.tile_pool(name="ps_w", bufs=2, space=PSUM) as ps_w,
        ):


            # ---- persistent SBUF tensors ----
            t_wq = pp.tile([128, IT, OC], FP16)
            t_wk = pp.tile([128, IT, OC], FP16)
            t_wv = pp.tile([128, IT, OC], FP16)
            t_wo = pp.tile([128, OT, D], FP16)
            t_k32 = pp.tile([128, 20 + OC], F32)
            t_k16 = pp.tile([128, KB * HPG + 256], FP16)
            t_qT = pp.tile([128, OT, S], FP16)
            t_kT = pp.tile([128, OT, KW], FP16)
            t_V = pp.tile([128, kb_cap, HPG, 128], FP16)
            t_OT = pp.tile([128, OT, S], FP16)

            t_xq = pp.tile([128, SC, IT, 512], FP16)
            t_xk = pp.tile([128, ksc, IT, 512], FP16)
            t_xv = pp.tile([128, VC, IT, 512], FP16)

            # ---- input DMAs; only SP/Activation/GpSimd queues can start DMAs.
            # Only the first chunk of each x stream moves upfront; later
            # chunks are enqueued mid-program (at flush points) so they don't
            # steal HBM bandwidth from the critical startup prefix.
            CW = IT * 512

            def load_x_chunk(eng, t_x, x_dram, c):
                eng.dma_start(
                    out=t_x[:, c, :, :],
                    in_=x_dram[:, c * CW:(c + 1) * CW].rearrange(
                        "p (i s) -> p i s", i=IT),
                )

            # gpsimd's library load blocks its queue ~11us, so it carries no
            # startup DMAs; sync/scalar split the critical prefix in
            # need-order: q bundle and k bundle first, then the v bundle.
            nc.gpsimd.load_library(library_config.attn)
            nc.sync.dma_start(out=t_wq, in_=wq[:].rearrange("p (i o) -> p i o", i=IT))
            load_x_chunk(nc.sync, t_xq, xq, 0)
            nc.scalar.dma_start(out=t_k32, in_=konst[:])
            nc.scalar.dma_start(out=t_k16, in_=konst16[:])
            nc.scalar.dma_start(out=t_wk, in_=wk[:].rearrange("p (i o) -> p i o", i=IT))
            load_x_chunk(nc.scalar, t_xk, xk, 0)
            load_x_chunk(nc.sync, t_xv, xv, 0)
            nc.scalar.dma_start(out=t_wv, in_=wv[:].rearrange("p (i o) -> p i o", i=IT))

            def load_later_chunks(qc):
                # called at flush(qc): bring in the chunks needed next
                c = qc + 1
                if c < SC:
                    load_x_chunk(nc.sync, t_xq, xq, c)
                if c < ksc:
                    load_x_chunk(nc.scalar, t_xk, xk, c)
                if c < VC:
                    load_x_chunk(nc.scalar if c == 1 else nc.gpsimd, t_xv, xv, c)
                if c == 1:  # wo needed once C(0) fillers start popping
                    nc.gpsimd.dma_start(
                        out=t_wo, in_=wo[:].rearrange("p (j d) -> p j d", j=OT))

            # early dummy exp: pull the ACT table load into the startup window
            nc.scalar.activation(
                out=t_OT[0:1, 0, 0:1], in_=t_k32[0:1, 0:1], func=Exp)

            # ---- work-unit generators ----
            def qk_proj_group(which, pair, sc):
                w_sb = t_wq if which == "q" else t_wk
                xts = t_xq if which == "q" else t_xk
                dst = t_qT if which == "q" else t_kT
                bidx = 0 if which == "q" else 1
                acc = ps_w.tile([128, 512], F32, tag="w",
                                name=f"acc_{which}_{pair}_{sc}")
                for i in range(IT):
                    nc.tensor.matmul(
                        acc,
                        w_sb[:, i, pair * 128:(pair + 1) * 128],
                        xts[:, sc, i, :],
                        start=(i == 0),
                        stop=(i == IT - 1),
                    )
                nc.vector.tensor_scalar_add(
                    out=dst[:, pair, sc * 512:(sc + 1) * 512],
                    in0=acc,
                    scalar1=t_k32[:, bidx * 2 + pair:bidx * 2 + pair + 1],
                )

            def v_proj_group(st):
                vacc = ps_w.tile([128, 512], F32, tag="w", name=f"vacc_{st}")
                for i in range(IT):
                    nc.tensor.matmul(
                        vacc[:, 0:OC],
                        t_xv[:, st // 4, i, (st % 4) * 128:(st % 4 + 1) * 128],
                        t_wv[:, i, :],
                        start=(i == 0),
                        stop=(i == IT - 1),
                    )
                tmp = vtp.tile([128, OC], F32, tag="vt", name=f"vt_{st}", bufs=2)
                nc.vector.tensor_add(out=tmp, in0=vacc[:, 0:OC], in1=t_k32[:, 20:20 + OC])
                nc.vector.tensor_scalar_mul(
                    out=t_V[:, st, :, 0:DK],
                    in0=tmp.rearrange("p (h d) -> p h d", h=HPG),
                    scalar1=t_k32[:, 4 + st:5 + st],
                )
                # ones-column of V = valid-key mask for this block
                nc.vector.tensor_copy(
                    t_V[:, st, :, DK:DK + 1],
                    t_k16[:, st * HPG:(st + 1) * HPG].rearrange(
                        "p (h o) -> p h o", o=1),
                )

            def c_group(qc, dt):
                q0 = qc * 512
                pc = ps_w.tile([128, 512], F32, tag="w", name=f"c_{qc}_{dt}")
                for j in range(OT):
                    nc.tensor.matmul(
                        pc,
                        t_wo[:, j, dt * 128:(dt + 1) * 128],
                        t_OT[:, j, q0:q0 + 512],
                        start=(j == 0),
                        stop=(j == OT - 1),
                    )
                so = stgp.tile([128, 512], FP16, tag="so", name=f"so_{qc}_{dt}")
                nc.vector.tensor_copy(so, pc)
                nc.sync.dma_start(
                    out=out_t[dt * 128:(dt + 1) * 128, q0:q0 + 512], in_=so)

            # filler queue: (deadline_qc, cost_estimate, fn)
            fill = []
            for sc in range(SC):
                for pair in range(OT):
                    fill.append((sc, 1.7, lambda p=pair, s=sc: qk_proj_group("q", p, s)))
            for sc in range(ksc):
                dl = max(0, (sc * 512) // 512)  # needed for attn(qc>=sc)
                for pair in range(OT):
                    fill.append((dl, 1.7, lambda p=pair, s=sc: qk_proj_group("k", p, s)))
            # v st<4 are issued inline inside attention(0, pair0) so the PE
            # can start QK/exp before the v x-chunk has landed
            n_inline_v = min(4, kb_cap)
            for st in range(n_inline_v, kb_cap):
                fill.append((st // 4, 0.9, lambda s=st: v_proj_group(s)))
            # order by deadline so flush/pacing pops prerequisites first
            fill.sort(key=lambda e: e[0])

            debt = [0.0]

            def maybe_fill(budget):
                debt[0] += budget
                while fill and debt[0] >= fill[0][1]:
                    _, cost, fn = fill.pop(0)
                    fn()
                    debt[0] -= cost

            def flush(qc):
                while fill and fill[0][0] <= qc:
                    _, _, fn = fill.pop(0)
                    fn()
                debt[0] = 0.0

            def attn_pair(qc, pair):
                q0 = qc * 512
                nkb = min(4 * (qc + 1), kb_cap)
                o_ps = [
                    ps_o.tile([128, 512], F32, tag="o", name=f"o_{qc}_{pair}_{a}")
                    for a in range(2)
                ]
                pts = {}

                def qk_exp(kb):
                    k0 = kb * 128
                    off = max(0, k0 - q0)
                    st = ps_st.tile([128, 1024], F32, tag="st",
                                    name=f"st_{qc}_{pair}_{kb}")
                    for a in range(2):
                        nc.tensor.matmul(
                            st[:, a * 512 + off:(a + 1) * 512],
                            t_kT[a * 64:(a + 1) * 64, pair, k0:k0 + 128],
                            t_qT[a * 64:(a + 1) * 64, pair, q0 + off:q0 + 512],
                            start=True,
                            stop=True,
                        )
                    if k0 >= q0:  # diagonal block: causal fix for both heads
                        for a in range(2):
                            nc.vector.tensor_add(
                                out=st[:, a * 512 + off:a * 512 + off + 128],
                                in0=st[:, a * 512 + off:a * 512 + off + 128],
                                in1=t_k16[:, KB * HPG:KB * HPG + 128],
                            )
                    pt = ptp.tile([128, 1024], FP16, tag="pt",
                                  name=f"pt_{qc}_{pair}_{kb}")
                    nc.scalar.activation(out=pt, in_=st, func=Exp)
                    pts[kb] = pt

                def pv(kb):
                    k0 = kb * 128
                    off = max(0, k0 - q0)
                    pt = pts.pop(kb)
                    for a in range(2):
                        nc.tensor.matmul(
                            o_ps[a][0:DK + 1, off:512],
                            t_V[:, kb, 2 * pair + a, 0:DK + 1],
                            pt[:, a * 512 + off:(a + 1) * 512],
                            start=(kb == 0),
                            stop=(kb == nkb - 1),
                        )

                inline_v = qc == 0 and pair == 0
                # during qc0 the later x-chunks are still in flight; popping
                # regular fillers would stall the PE on their DMAs
                budget = 0.0 if qc == 0 else 0.65
                qk_exp(0)
                if inline_v:
                    v_proj_group(0)
                for kb in range(1, nkb):
                    qk_exp(kb)
                    if inline_v and kb < n_inline_v:
                        v_proj_group(kb)
                    maybe_fill(budget)
                    pv(kb - 1)
                maybe_fill(budget)
                pv(nkb - 1)

                for a in range(2):
                    # copy out of PSUM promptly (two base-0 pieces) so the
                    # o_ps bank frees for the next pair; norm runs from SBUF
                    t_l = nrmp.tile([1, 512], F32, tag="l",
                                    name=f"l_{qc}_{pair}_{a}", bufs=3)
                    nc.vector.tensor_copy(t_l, o_ps[a][DK:DK + 1, :])
                    o_sb = nrmp.tile([DK, 512], F32, tag="osb",
                                     name=f"osb_{qc}_{pair}_{a}", bufs=3)
                    nc.vector.tensor_copy(o_sb, o_ps[a][0:DK, :])
                    r = nrmp.tile([1, 512], F32, tag="r", name=f"r_{qc}_{pair}_{a}")
                    nc.vector.reciprocal_approx_fast(r, t_l)
                    rb = nrmp.tile([DK, 512], F32, tag="rb",
                                   name=f"rb_{qc}_{pair}_{a}")
                    nc.gpsimd.partition_broadcast(rb, r)
                    nc.vector.tensor_mul(
                        t_OT[a * 64:(a + 1) * 64, pair, q0:q0 + 512],
                        o_sb,
                        rb,
                    )

            # ---- main pipeline ----
            for qc in range(SC):
                load_later_chunks(qc)
                flush(qc)
                for pair in range(OT):
                    attn_pair(qc, pair)
                for dt in range(D // 128):
                    fill.append((SC + 1, 0.9, lambda q=qc, d=dt: c_group(q, d)))
            while fill:
                fill.pop(0)[2]()

    nc.compile()
    return nc


def _get_nc(kb_cap):
    key = ("nc", kb_cap)
    if key not in _cache:
        _cache[key] = _build_nc(kb_cap)
    return _cache[key]


def kernel(
    query,
    key,
    value,
    Wq,
    bq,
    Wk,
    bk,
    Wv,
    bv,
    Wo,
    bo,
    attn_mask,
    key_padding_mask,
):
    from concourse import bass_utils

    query = np.asarray(query, dtype=np.float32)
    key = np.asarray(key, dtype=np.float32)
    value = np.asarray(value, dtype=np.float32)
    Wq = np.asarray(Wq, dtype=np.float32)
    bq = np.asarray(bq, dtype=np.float32)
    Wk = np.asarray(Wk, dtype=np.float32)
    bk = np.asarray(bk, dtype=np.float32)
    Wv = np.asarray(Wv, dtype=np.float32)
    bv = np.asarray(bv, dtype=np.float32)
    Wo = np.asarray(Wo, dtype=np.float32)
    bo = np.asarray(bo, dtype=np.float32)
    attn_mask = np.asarray(attn_mask)
    key_padding_mask = np.asarray(key_padding_mask)

    # this kernel hardcodes the causal structure of attn_mask
    expected = np.triu(np.ones((S, S), dtype=bool), k=1)
    assert np.array_equal(attn_mask, expected), "kernel assumes causal attn_mask"

    # number of 128-blocks that contain any valid (unpadded) key
    valid = ~key_padding_mask  # [B, S]
    kb_cap = 0
    for b in range(B):
        nz = np.nonzero(valid[b])[0]
        cap = (int(nz.max()) // 128 + 1) if nz.size else 1
        kb_cap = max(kb_cap, cap)

    scale = np.float32(1.0 / np.sqrt(DK))
    ctile = np.where(
        np.arange(128)[None, :] >= np.arange(128)[:, None], 0.0, -60000.0
    ).astype(np.float16)
    causal2 = np.ascontiguousarray(np.concatenate([ctile, ctile], axis=1))

    def pack_w(w):  # [D, OC] -> [128, IT*OC] p-major
        return np.ascontiguousarray(
            w.reshape(IT, 128, OC).transpose(1, 0, 2).reshape(128, IT * OC)
        ).astype(np.float16)

    # per-batch transposed activations (shared by the batch's 4 cores),
    # packed chunk-major [128, chunks*IT*512] to match the SBUF layout
    ksc = -(-kb_cap * 128 // 512)
    VC = -(-kb_cap // 4)

    def pack_x(x, b, nchunks):  # x [S, B, D] -> [128, nchunks*IT*512]
        xt = x[:, b, :].T.astype(np.float16)  # [D, S]
        xt = xt[:, 0:nchunks * 512]
        return np.ascontiguousarray(
            xt.reshape(IT, 128, nchunks, 512).transpose(1, 2, 0, 3)
            .reshape(128, nchunks * IT * 512))

    xq_b = [pack_x(query, b, SC) for b in range(B)]
    xk_b = [pack_x(key, b, ksc) for b in range(B)]
    xv_b = [pack_x(value, b, VC) for b in range(B)]
    vm_b = [valid[b].astype(np.float32).reshape(KB, 128).T for b in range(B)]

    in_maps = []
    for c in range(N_CORES):
        b = c // GROUPS
        g = c % GROUPS
        o0 = g * OC
        osl = slice(o0, o0 + OC)
        konst = np.zeros((128, 20 + OC), np.float32)
        konst[:, 0:OT] = (bq[osl] * scale).reshape(OT, 128).T
        konst[:, 2:2 + OT] = bk[osl].reshape(OT, 128).T
        konst[:, 4:4 + KB] = vm_b[b]
        konst[:, 20:] = bv[osl][None, :]
        konst16 = np.zeros((128, KB * HPG + 256), np.float16)
        konst16[:, 0:KB * HPG] = np.repeat(
            vm_b[b].astype(np.float16)[:, :, None], HPG, axis=2
        ).reshape(128, KB * HPG)
        konst16[:, KB * HPG:] = causal2
        in_maps.append(
            {
                "xq": xq_b[b],
                "xk": xk_b[b],
                "xv": xv_b[b],
                "wq": pack_w((Wq[osl, :] * scale).T),
                "wk": pack_w(Wk[osl, :].T),
                "wv": pack_w(Wv[osl, :].T),
                "wo": np.ascontiguousarray(
                    Wo[:, osl].T.reshape(OT, 128, D).transpose(1, 0, 2)
                    .reshape(128, OT * D)).astype(np.float16),
                "konst": np.ascontiguousarray(konst),
                "konst16": np.ascontiguousarray(konst16),
            }
        )

    res = bass_utils.run_bass_kernel_spmd(
        _get_nc(kb_cap), in_maps, core_ids=list(range(N_CORES))
    )
    _cache["last_res"] = res

    out = np.zeros((S, B, D), dtype=np.float32)
    for b in range(B):
        acc = np.zeros((D, S), dtype=np.float32)
        for g in range(GROUPS):
            acc += res.results[b * GROUPS + g]["out_t"].astype(np.float32)
        out[:, b, :] = acc.T + bo[None, :]
    return out


# revision 46
# speedup vs baseline: 1.1660x; 1.0292x over previous
"""Trainium2 Bass kernel for nn_MultiHeadAttention_59614146068609.

Sharding: 8 cores = 2 batches x 4 head-groups (4 heads each). Each core
projects q/k/v for its batch with its head-slice of Wq/Wk/Wv
(column-sharded), runs causal+padded attention for its 4 heads, and
applies its row-slice of Wo, producing a partial [D, S] fp16 output.
The host sums the 4 partials per batch and adds bo.

Schedule: single software-pipelined pass. Attention is ACT(exp)-paced,
so projection and output (Wo) matmul groups are injected as PE filler
between attention steps; the PE stays busy while the scalar engine
churns exp.

Key layout choices:
 - scores computed transposed (S.T[k, q], k on partitions); softmax
   denominator comes from an appended ones-column of V.
 - heads processed in pairs: qT/kT hold a head pair stacked on
   partitions (64+64); QK runs as two concurrent row-tiled matmuls
   (tile_position (0,0)/(64,0)), scores for the pair land in one
   2-bank PSUM tile and one ACTIVATE(exp) covers both heads.
 - key-padding folded into V: padded V rows are zeroed and the
   ones-column holds the valid mask, so exp needs no per-block bias
   and masked keys contribute exactly zero weight and zero denominator.

Specialized at build time on kb_cap = number of 128-wide key blocks
containing any unpadded key.
"""

import numpy as np

S = 2048
B = 2
D = 1024
H = 16
DK = 64
N_CORES = 8
GROUPS = N_CORES // B          # head groups per batch = 4
HPG = H // GROUPS              # heads per group = 4
OC = HPG * DK                  # per-core projected dim = 256
OT = OC // 128                 # head pairs per core = 2
IT = D // 128                  # contraction tiles = 8
SC = S // 512                  # sequence chunks of 512 = 4
KB = S // 128                  # k blocks of 128 = 16

_cache = {}


def _build_nc(kb_cap):
    import concourse.bacc as bacc
    import concourse.bass as bass
    import concourse.mybir as mybir
    import concourse.tile as tile
    from concourse import library_config

    F32 = mybir.dt.float32
    FP16 = mybir.dt.float16
    FP8 = mybir.dt.float8e4
    Exp = mybir.ActivationFunctionType.Exp
    PSUM = bass.MemorySpace.PSUM

    ksc = -(-kb_cap * 128 // 512)        # 512-chunks of k to project
    KW = ksc * 512
    VW = kb_cap * 128

    VC = -(-kb_cap // 4)                 # 512-wide chunks of v keys

    nc = bacc.Bacc("TRN2", target_bir_lowering=False, debug=False)

    # x streams pre-packed chunk-major on host: [128, chunk, IT, 512]
    xq = nc.dram_tensor("xq", [128, SC * IT * 512], FP16, kind="ExternalInput")
    xk = nc.dram_tensor("xk", [128, ksc * IT * 512], FP16, kind="ExternalInput")
    xv = nc.dram_tensor("xv", [128, VC * IT * 512], FP16, kind="ExternalInput")
    wq = nc.dram_tensor("wq", [128, IT * OC], FP16, kind="ExternalInput")
    wk = nc.dram_tensor("wk", [128, IT * OC], FP16, kind="ExternalInput")
    wv = nc.dram_tensor("wv", [128, IT * OC], FP16, kind="ExternalInput")
    wo = nc.dram_tensor("wo", [128, OT * D], FP16, kind="ExternalInput")
    # konst f32: [0:2]=bias_q(pair), [2:4]=bias_k, [4:20]=vmask, [20:276]=bv
    konst = nc.dram_tensor("konst", [128, 20 + OC], F32, kind="ExternalInput")
    # konst16 fp16: [0:64]=vmask4 (st-major), [64:320]=causal2
    konst16 = nc.dram_tensor("konst16", [128, KB * HPG + 256], FP16,
                             kind="ExternalInput")
    out_t = nc.dram_tensor("out_t", [D, S], FP16, kind="ExternalOutput")

    with tile.TileContext(nc) as tc, nc.allow_low_precision(
        reason="fp16 compute throughout; validated vs fp64 reference"
    ):
        with (
            tc.tile_pool(name="persist", bufs=1) as pp,
            tc.tile_pool(name="pt", bufs=3) as ptp,
            tc.tile_pool(name="nrm", bufs=2) as nrmp,
            tc.tile_pool(name="stg", bufs=3) as stgp,
            tc.tile_pool(name="vtmp", bufs=2) as vtp,
            tc.tile_pool(name="ps_st", bufs=2, space=PSUM) as ps_st,
            tc.tile_pool(name="ps_o", bufs=2, space=PSUM) as ps_o,
            tc.tile_pool(name="ps_w", bufs=2, space=PSUM) as ps_w,
        ):


            # ---- persistent SBUF tensors ----
            t_wq = pp.tile([128, IT, OC], FP16)
            t_wk = pp.tile([128, IT, OC], FP16)
            t_wv = pp.tile([128, IT, OC], FP16)
            t_wo = pp.tile([128, OT, D], FP16)
            t_k32 = pp.tile([128, 20 + OC], F32)
            t_k16 = pp.tile([128, KB * HPG + 256], FP16)
            t_qT = pp.tile([128, OT, S], FP16)
            t_kT = pp.tile([128, OT, KW], FP16)
            t_V = pp.tile([128, kb_cap, HPG, 128], FP16)
            t_OT = pp.tile([128, OT, S], FP16)

            t_xq = pp.tile([128, SC, IT, 512], FP16)
            t_xk = pp.tile([128, ksc, IT, 512], FP16)
            t_xv = pp.tile([128, VC, IT, 512], FP16)

            # ---- input DMAs; only SP/Activation/GpSimd queues can start DMAs.
            # Only the first chunk of each x stream moves upfront; later
            # chunks are enqueued mid-program (at flush points) so they don't
            # steal HBM bandwidth from the critical startup prefix.
            CW = IT * 512

            def load_x_chunk(eng, t_x, x_dram, c, split=1):
                ih = IT // split
                for h in range(split):
                    eng.dma_start(
                        out=t_x[:, c, h * ih:(h + 1) * ih, :],
                        in_=x_dram[:, c * CW + h * ih * 512:
                                   c * CW + (h + 1) * ih * 512].rearrange(
                            "p (i s) -> p i s", i=ih),
                    )

            # gpsimd's library load blocks its queue ~11us, so it carries no
            # startup DMAs; sync/scalar split the critical prefix in
            # need-order: q bundle and k bundle first, then the v bundle.
            nc.gpsimd.load_library(library_config.attn)
            nc.sync.dma_start(out=t_wq, in_=wq[:].rearrange("p (i o) -> p i o", i=IT))
            load_x_chunk(nc.sync, t_xq, xq, 0, split=2)
            nc.scalar.dma_start(out=t_k32, in_=konst[:])
            nc.scalar.dma_start(out=t_k16, in_=konst16[:])
            nc.scalar.dma_start(out=t_wk, in_=wk[:].rearrange("p (i o) -> p i o", i=IT))
            load_x_chunk(nc.scalar, t_xk, xk, 0, split=2)
            load_x_chunk(nc.sync, t_xv, xv, 0, split=2)
            nc.scalar.dma_start(out=t_wv, in_=wv[:].rearrange("p (i o) -> p i o", i=IT))

            def load_later_chunks(qc):
                # called at flush(qc): bring in the chunks needed next
                c = qc + 1
                if c < SC:
                    load_x_chunk(nc.sync, t_xq, xq, c)
                if c < ksc:
                    load_x_chunk(nc.scalar, t_xk, xk, c)
                if c < VC:
                    load_x_chunk(nc.scalar if c == 1 else nc.gpsimd, t_xv, xv, c)
                if c == 1:  # wo needed once C(0) fillers start popping
                    nc.gpsimd.dma_start(
                        out=t_wo, in_=wo[:].rearrange("p (j d) -> p j d", j=OT))

            # early dummy exp: pull the ACT table load into the startup window
            nc.scalar.activation(
                out=t_OT[0:1, 0, 0:1], in_=t_k32[0:1, 0:1], func=Exp)

            # ---- work-unit generators ----
            def qk_proj_group(which, pair, sc):
                w_sb = t_wq if which == "q" else t_wk
                xts = t_xq if which == "q" else t_xk
                dst = t_qT if which == "q" else t_kT
                bidx = 0 if which == "q" else 1
                acc = ps_w.tile([128, 512], F32, tag="w",
                                name=f"acc_{which}_{pair}_{sc}")
                for i in range(IT):
                    nc.tensor.matmul(
                        acc,
                        w_sb[:, i, pair * 128:(pair + 1) * 128],
                        xts[:, sc, i, :],
                        start=(i == 0),
                        stop=(i == IT - 1),
                    )
                nc.vector.tensor_scalar_add(
                    out=dst[:, pair, sc * 512:(sc + 1) * 512],
                    in0=acc,
                    scalar1=t_k32[:, bidx * 2 + pair:bidx * 2 + pair + 1],
                )

            def v_proj_group(st):
                vacc = ps_w.tile([128, 512], F32, tag="w", name=f"vacc_{st}")
                for i in range(IT):
                    nc.tensor.matmul(
                        vacc[:, 0:OC],
                        t_xv[:, st // 4, i, (st % 4) * 128:(st % 4 + 1) * 128],
                        t_wv[:, i, :],
                        start=(i == 0),
                        stop=(i == IT - 1),
                    )
                tmp = vtp.tile([128, OC], F32, tag="vt", name=f"vt_{st}", bufs=2)
                nc.vector.tensor_add(out=tmp, in0=vacc[:, 0:OC], in1=t_k32[:, 20:20 + OC])
                nc.vector.tensor_scalar_mul(
                    out=t_V[:, st, :, 0:DK],
                    in0=tmp.rearrange("p (h d) -> p h d", h=HPG),
                    scalar1=t_k32[:, 4 + st:5 + st],
                )
                # ones-column of V = valid-key mask for this block
                nc.vector.tensor_copy(
                    t_V[:, st, :, DK:DK + 1],
                    t_k16[:, st * HPG:(st + 1) * HPG].rearrange(
                        "p (h o) -> p h o", o=1),
                )

            def c_group(qc, dt):
                q0 = qc * 512
                pc = ps_w.tile([128, 512], F32, tag="w", name=f"c_{qc}_{dt}")
                for j in range(OT):
                    nc.tensor.matmul(
                        pc,
                        t_wo[:, j, dt * 128:(dt + 1) * 128],
                        t_OT[:, j, q0:q0 + 512],
                        start=(j == 0),
                        stop=(j == OT - 1),
                    )
                so = stgp.tile([128, 512], FP16, tag="so", name=f"so_{qc}_{dt}")
                nc.vector.tensor_copy(so, pc)
                nc.sync.dma_start(
                    out=out_t[dt * 128:(dt + 1) * 128, q0:q0 + 512], in_=so)

            # filler queue: (deadline_qc, cost_estimate, fn)
            fill = []
            for sc in range(SC):
                for pair in range(OT):
                    fill.append((sc, 1.7, lambda p=pair, s=sc: qk_proj_group("q", p, s)))
            for sc in range(ksc):
                dl = max(0, (sc * 512) // 512)  # needed for attn(qc>=sc)
                for pair in range(OT):
                    fill.append((dl, 1.7, lambda p=pair, s=sc: qk_proj_group("k", p, s)))
            # v st<4 are issued inline inside attention(0, pair0) so the PE
            # can start QK/exp before the v x-chunk has landed
            n_inline_v = min(4, kb_cap)
            for st in range(n_inline_v, kb_cap):
                fill.append((st // 4, 0.9, lambda s=st: v_proj_group(s)))
            # order by deadline so flush/pacing pops prerequisites first
            fill.sort(key=lambda e: e[0])

            debt = [0.0]

            def maybe_fill(budget):
                debt[0] += budget
                while fill and debt[0] >= fill[0][1]:
                    _, cost, fn = fill.pop(0)
                    fn()
                    debt[0] -= cost

            def flush(qc):
                while fill and fill[0][0] <= qc:
                    _, _, fn = fill.pop(0)
                    fn()
                debt[0] = 0.0

            def attn_pair(qc, pair):
                q0 = qc * 512
                nkb = min(4 * (qc + 1), kb_cap)
                o_ps = [
                    ps_o.tile([128, 512], F32, tag="o", name=f"o_{qc}_{pair}_{a}")
                    for a in range(2)
                ]
                pts = {}

                def qk_exp(kb):
                    k0 = kb * 128
                    off = max(0, k0 - q0)
                    st = ps_st.tile([128, 1024], F32, tag="st",
                                    name=f"st_{qc}_{pair}_{kb}")
                    for a in range(2):
                        nc.tensor.matmul(
                            st[:, a * 512 + off:(a + 1) * 512],
                            t_kT[a * 64:(a + 1) * 64, pair, k0:k0 + 128],
                            t_qT[a * 64:(a + 1) * 64, pair, q0 + off:q0 + 512],
                            start=True,
                            stop=True,
                        )
                    if k0 >= q0:  # diagonal block: causal fix for both heads
                        for a in range(2):
                            nc.vector.tensor_add(
                                out=st[:, a * 512 + off:a * 512 + off + 128],
                                in0=st[:, a * 512 + off:a * 512 + off + 128],
                                in1=t_k16[:, KB * HPG:KB * HPG + 128],
                            )
                    pt = ptp.tile([128, 1024], FP16, tag="pt",
                                  name=f"pt_{qc}_{pair}_{kb}")
                    nc.scalar.activation(out=pt, in_=st, func=Exp)
                    pts[kb] = pt

                def pv(kb):
                    k0 = kb * 128
                    off = max(0, k0 - q0)
                    pt = pts.pop(kb)
                    for a in range(2):
                        nc.tensor.matmul(
                            o_ps[a][0:DK + 1, off:512],
                            t_V[:, kb, 2 * pair + a, 0:DK + 1],
                            pt[:, a * 512 + off:(a + 1) * 512],
                            start=(kb == 0),
                            stop=(kb == nkb - 1),
                        )

                inline_v = qc == 0 and pair == 0
                # pair0 of qc0 is fed by the inline v groups; its x-chunk is
                # still landing, so no regular pops there
                budget = 0.0 if inline_v else (0.56 if qc == 0 else 0.6)
                qk_exp(0)
                for kb in range(1, nkb):
                    qk_exp(kb)
                    if inline_v:
                        v_proj_group(kb - 1)
                    maybe_fill(budget)
                    pv(kb - 1)
                if inline_v:
                    v_proj_group(nkb - 1)
                maybe_fill(budget)
                pv(nkb - 1)

                for a in range(2):
                    # copy out of PSUM promptly (two base-0 pieces) so the
                    # o_ps bank frees for the next pair; norm runs from SBUF
                    t_l = nrmp.tile([1, 512], F32, tag="l",
                                    name=f"l_{qc}_{pair}_{a}", bufs=3)
                    nc.vector.tensor_copy(t_l, o_ps[a][DK:DK + 1, :])
                    o_sb = nrmp.tile([DK, 512], F32, tag="osb",
                                     name=f"osb_{qc}_{pair}_{a}", bufs=3)
                    nc.vector.tensor_copy(o_sb, o_ps[a][0:DK, :])
                    r = nrmp.tile([1, 512], F32, tag="r", name=f"r_{qc}_{pair}_{a}")
                    nc.vector.reciprocal_approx_fast(r, t_l)
                    rb = nrmp.tile([DK, 512], F32, tag="rb",
                                   name=f"rb_{qc}_{pair}_{a}")
                    nc.gpsimd.partition_broadcast(rb, r)
                    nc.vector.tensor_mul(
                        t_OT[a * 64:(a + 1) * 64, pair, q0:q0 + 512],
                        o_sb,
                        rb,
                    )

            # ---- main pipeline ----
            for qc in range(SC):
                load_later_chunks(qc)
                flush(qc)
                for pair in range(OT):
                    attn_pair(qc, pair)
                for dt in range(D // 128):
                    fill.append((SC + 1, 0.9, lambda q=qc, d=dt: c_group(q, d)))
            while fill:
                fill.pop(0)[2]()

    nc.compile()
    return nc


def _get_nc(kb_cap):
    key = ("nc", kb_cap)
    if key not in _cache:
        _cache[key] = _build_nc(kb_cap)
    return _cache[key]


def kernel(
    query,
    key,
    value,
    Wq,
    bq,
    Wk,
    bk,
    Wv,
    bv,
    Wo,
    bo,
    attn_mask,
    key_padding_mask,
):
    from concourse import bass_utils

    query = np.asarray(query, dtype=np.float32)
    key = np.asarray(key, dtype=np.float32)
    value = np.asarray(value, dtype=np.float32)
    Wq = np.asarray(Wq, dtype=np.float32)
    bq = np.asarray(bq, dtype=np.float32)
    Wk = np.asarray(Wk, dtype=np.float32)
    bk = np.asarray(bk, dtype=np.float32)
    Wv = np.asarray(Wv, dtype=np.float32)
    bv = np.asarray(bv, dtype=np.float32)
    Wo = np.asarray(Wo, dtype=np.float32)
    bo = np.asarray(bo, dtype=np.float32)
    attn_mask = np.asarray(attn_mask)
    key_padding_mask = np.asarray(key_padding_mask)

    # this kernel hardcodes the causal structure of attn_mask
    expected = np.triu(np.ones((S, S), dtype=bool), k=1)
    assert np.array_equal(attn_mask, expected), "kernel assumes causal attn_mask"

    # number of 128-blocks that contain any valid (unpadded) key
    valid = ~key_padding_mask  # [B, S]
    kb_cap = 0
    for b in range(B):
        nz = np.nonzero(valid[b])[0]
        cap = (int(nz.max()) // 128 + 1) if nz.size else 1
        kb_cap = max(kb_cap, cap)

    scale = np.float32(1.0 / np.sqrt(DK))
    ctile = np.where(
        np.arange(128)[None, :] >= np.arange(128)[:, None], 0.0, -60000.0
    ).astype(np.float16)
    causal2 = np.ascontiguousarray(np.concatenate([ctile, ctile], axis=1))

    def pack_w(w):  # [D, OC] -> [128, IT*OC] p-major
        return np.ascontiguousarray(
            w.reshape(IT, 128, OC).transpose(1, 0, 2).reshape(128, IT * OC)
        ).astype(np.float16)

    # per-batch transposed activations (shared by the batch's 4 cores),
    # packed chunk-major [128, chunks*IT*512] to match the SBUF layout
    ksc = -(-kb_cap * 128 // 512)
    VC = -(-kb_cap // 4)

    def pack_x(x, b, nchunks):  # x [S, B, D] -> [128, nchunks*IT*512]
        xt = x[:, b, :].T.astype(np.float16)  # [D, S]
        xt = xt[:, 0:nchunks * 512]
        return np.ascontiguousarray(
            xt.reshape(IT, 128, nchunks, 512).transpose(1, 2, 0, 3)
            .reshape(128, nchunks * IT * 512))

    xq_b = [pack_x(query, b, SC) for b in range(B)]
    xk_b = [pack_x(key, b, ksc) for b in range(B)]
    xv_b = [pack_x(value, b, VC) for b in range(B)]
    vm_b = [valid[b].astype(np.float32).reshape(KB, 128).T for b in range(B)]

    in_maps = []
    for c in range(N_CORES):
        b = c // GROUPS
        g = c % GROUPS
        o0 = g * OC
        osl = slice(o0, o0 + OC)
        konst = np.zeros((128, 20 + OC), np.float32)
        konst[:, 0:OT] = (bq[osl] * scale).reshape(OT, 128).T
        konst[:, 2:2 + OT] = bk[osl].reshape(OT, 128).T
        konst[:, 4:4 + KB] = vm_b[b]
        konst[:, 20:] = bv[osl][None, :]
        konst16 = np.zeros((128, KB * HPG + 256), np.float16)
        konst16[:, 0:KB * HPG] = np.repeat(
            vm_b[b].astype(np.float16)[:, :, None], HPG, axis=2
        ).reshape(128, KB * HPG)
        konst16[:, KB * HPG:] = causal2
        in_maps.append(
            {
                "xq": xq_b[b],
                "xk": xk_b[b],
                "xv": xv_b[b],
                "wq": pack_w((Wq[osl, :] * scale).T),
                "wk": pack_w(Wk[osl, :].T),
                "wv": pack_w(Wv[osl, :].T),
                "wo": np.ascontiguousarray(
                    Wo[:, osl].T.reshape(OT, 128, D).transpose(1, 0, 2)
                    .reshape(128, OT * D)).astype(np.float16),
                "konst": np.ascontiguousarray(konst),
                "konst16": np.ascontiguousarray(konst16),
            }
        )

    res = bass_utils.run_bass_kernel_spmd(
        _get_nc(kb_cap), in_maps, core_ids=list(range(N_CORES))
    )
    _cache["last_res"] = res

    out = np.zeros((S, B, D), dtype=np.float32)
    for b in range(B):
        acc = np.zeros((D, S), dtype=np.float32)
        for g in range(GROUPS):
            acc += res.results[b * GROUPS + g]["out_t"].astype(np.float32)
        out[:, b, :] = acc.T + bo[None, :]
    return out
